# revision 9
# baseline (speedup 1.0000x reference)
"""Depthwise causal Conv1D (B=4, C=4096, L=4096, K=4) on 8 trn2 NeuronCores.

Sharding: channel-parallel (tensor parallel) — core i owns channels
[i*512, (i+1)*512). Depthwise conv has zero cross-channel interaction, so
there is no communication; each core computes its channel slab end to end.

Per-core kernel layout: channels on SBUF partitions (128 at a time), time on
the free dim. The 4-tap causal FIR along the free dim is computed as four
shifted multiply-accumulate passes with per-partition (per-channel) scalar
weights, split across three engines so no single engine is the bottleneck:

  ScalarE : out[3:L+3]  = w0 * x + bias   (activation, per-partition scale+bias)
            out[0:3]    = bias
  GPSIMD  : out[0:L]   += w3 * x          (scalar_tensor_tensor)
  VectorE : out[1:L+1] += w2 * x          (scalar_tensor_tensor)
            out[2:L+2] += w1 * x          (scalar_tensor_tensor)

DMA (HWDGE via nc.sync) streams 128x4096 fp32 tiles in and 128x4099 tiles
out; the kernel is HBM-bandwidth bound (~64 MB per core total traffic).
"""

import numpy as np

import concourse.bass as bass
import concourse.tile as tile
from concourse import bacc, mybir
from concourse.bass_utils import run_bass_kernel_spmd

B, C, L, K = 4, 4096, 4096, 4
PAD = K - 1
LOUT = L + PAD  # 4099
NCORES = 8
CS = C // NCORES  # 512 channels per core
DT = mybir.dt.float32

_AF = mybir.ActivationFunctionType
_OP = mybir.AluOpType


def build_nc(b=B, cs=CS, l=L, k=K, n_bufs=3):
    """Build the per-core Bass program. Parameterized for small-size sim tests."""
    ng = cs // 128
    lout = l + k - 1
    pad = k - 1

    nc = bacc.Bacc("TRN2", target_bir_lowering=False, debug=False, num_devices=NCORES)
    x_d = nc.dram_tensor("x", [b, cs, l], DT, kind="ExternalInput").ap()
    # packed per-channel constants: wb[c] = [w_0..w_{k-1}, bias]
    wb_d = nc.dram_tensor("wb", [cs, k + 1], DT, kind="ExternalInput").ap()
    o_d = nc.dram_tensor("out", [b, cs, lout], DT, kind="ExternalOutput").ap()

    with tile.TileContext(nc) as tc:
        with (
            tc.tile_pool(name="consts", bufs=1) as cpool,
            tc.tile_pool(name="xs", bufs=n_bufs) as xpool,
            tc.tile_pool(name="os", bufs=n_bufs + 1) as opool,
            tc.tile_pool(name="ts", bufs=n_bufs) as tpool,
        ):
            # Per-group constant columns: [128, k+1] = w_0..w_{k-1}, bias.
            consts = []
            for g in range(ng):
                ct = cpool.tile([128, k + 1], DT, tag=f"c{g}")
                nc.sync.dma_start(ct[:], wb_d[g * 128 : (g + 1) * 128, :])
                consts.append(ct)

            for bi in range(b):
                for g in range(ng):
                    ct = consts[g]
                    c0 = g * 128
                    xt = xpool.tile([128, l], DT, tag="x")
                    nc.sync.dma_start(xt[:], x_d[bi, c0 : c0 + 128, :])
                    ot = opool.tile([128, lout], DT, tag="o")

                    # tap 0 (+bias): out[pad:lout] = w0*x + bias  (ScalarE)
                    nc.scalar.activation(
                        ot[:, pad:lout], xt[:], _AF.Identity,
                        bias=ct[:, k : k + 1], scale=ct[:, 0:1],
                    )
                    # head columns [0:pad] = bias  (ScalarE)
                    nc.scalar.activation(
                        ot[:, 0:pad], xt[:, 0:pad], _AF.Identity,
                        bias=ct[:, k : k + 1], scale=0.0,
                    )
                    # tap k-1 via temp: t = w_{k-1} * x (ScalarE), out[0:l] += t (GPSIMD)
                    tt = tpool.tile([128, l], DT, tag="t")
                    nc.scalar.activation(
                        tt[:], xt[:], _AF.Copy, bias=0.0, scale=ct[:, k - 1 : k],
                    )
                    nc.gpsimd.tensor_tensor(
                        out=ot[:, 0:l], in0=ot[:, 0:l], in1=tt[:], op=_OP.add,
                    )
                    # middle taps on VectorE: tap t -> out[pad-t : pad-t+l] += w_t * x
                    for t in range(1, k - 1):
                        s = pad - t
                        nc.vector.scalar_tensor_tensor(
                            out=ot[:, s : s + l], in0=xt[:], scalar=ct[:, t : t + 1],
                            in1=ot[:, s : s + l], op0=_OP.mult, op1=_OP.add,
                        )
                    nc.sync.dma_start(o_d[bi, c0 : c0 + 128, :], ot[:])
    nc.compile()
    return nc


_cached_nc = None


def _get_nc():
    global _cached_nc
    if _cached_nc is None:
        _cached_nc = build_nc()
    return _cached_nc


def run(x, kernel, bias, trace=False, **kwargs):
    """Shard, run on 8 cores, gather. Returns (out, BassKernelResults)."""
    x = np.ascontiguousarray(x, dtype=np.float32)
    w = np.asarray(kernel, dtype=np.float32).reshape(K, C)
    bvec = np.asarray(bias, dtype=np.float32).reshape(C)
    # wb[c] = [w_0[c] .. w_{K-1}[c], bias[c]]
    wb = np.concatenate([w.T, bvec[:, None]], axis=1).astype(np.float32)

    in_maps = []
    for i in range(NCORES):
        sl = slice(i * CS, (i + 1) * CS)
        in_maps.append(
            {
                "x": np.ascontiguousarray(x[:, sl, :]),
                "wb": np.ascontiguousarray(wb[sl, :]),
            }
        )

    nc = _get_nc()
    bkr = run_bass_kernel_spmd(
        nc, in_maps, core_ids=list(range(NCORES)), trace=trace, **kwargs
    )
    out = np.concatenate([r["out"] for r in bkr.results], axis=1)
    return out, bkr


def kernel(x, kernel, bias):
    out, _ = run(x, kernel, bias)
    return out
